# revision 15
# baseline (speedup 1.0000x reference)
"""Cross-attention kernel for 8 Trainium2 NeuronCores (SPMD).

Problem: B=4, T_q=T_kv=2048, Q_DIM=1024, KV_DIM=768, H=16, DK=64, fp32.
  q = q_tokens @ Wq.T ; k = kv_tokens @ Wk.T ; v = kv_tokens @ Wv.T
  out = softmax(q k^T / sqrt(DK)) v @ Wo.T

Sharding (8 cores): core c handles batch b=c//2 and q-token half
qh=c%2 (1024 of the 2048 query tokens) with ALL 16 heads resident.
No collectives at all: the output projection contracts over the full
1024 attention-output dims locally, and the host concatenates the
per-core [1024 tokens, 1024 dims] blocks.  (K/V projections are
recomputed per batch-pair — far cheaper than pair AllGathers, whose
~15us setup at ~40GB/s would dominate the whole kernel.)

Everything stays in SBUF between phases (no K/V/Q DRAM round-trips):
inputs and weights stream in as bf16; Q/K/V are projected into
SBUF-resident bf16 tensors; attention runs per head-pair out of SBUF.
All matmuls take bf16 operands with fp32 PSUM accumulation.

Attention inner loop per (head-pair hp, 512-query block j, kv-chunk i):
scores for the pair's two heads go to the two 64-row halves of the PE
array (row tiling via base_partition/tile_position); exp runs on the
scalar engine (the only engine with activation tables — it is the
attention-phase bottleneck at ~1.04us per [128,1024] tile, so all DMA
issue is kept off it); the PV matmul is reoriented (exp'd scores
stationary [kv,128q], V moving [kv,65]) which halves its PE rows vs the
classic orientation, with the softmax denominator accumulated for free
by a ones-column appended to V.  Normalisation is a per-partition
reciprocal+scale on DVE (no partition broadcast), and the [q,dv]->[dv,q]
transpose rides the DMA xbar (dma_start_transpose) instead of the PE.

The remaining head-pairs' K/Q projections are interleaved unit-by-unit
into the attention stream so the PE fills the slack of the Act-bound
phase (front-loaded at block starts, which also hides the single-
buffered PSUM accumulator hand-off), and the output projection for the
first query block interleaves into the last attention block.

PSUM (8 banks): scores [P,1024]x2 = 4, accumulators [P,512]x2 = 2
(four 65-wide PV chains at 128-col slots per tile), projection/warm-up/
out-proj chains [P,512]x2 = 2.
"""

import numpy as np

import concourse.bacc as bacc
import concourse.mybir as mybir
import concourse.tile as tile
from concourse import bass_utils

N_CORES = 8
P = 128
TQC = 1024     # per-core query tokens
TKV = 2048
CQ = 1024      # q_tokens channels / q dims / output dims
CKV = 768      # kv_tokens channels
NHP = 8        # head-pairs (128 dims each)
NI = TKV // P  # 16 kv chunks
NJ = TQC // 512  # 2 query blocks
CQ_CH = CQ // P   # 8
CKV_CH = CKV // P  # 6

F32 = mybir.dt.float32
BF16 = mybir.dt.bfloat16
EXP = mybir.ActivationFunctionType.Exp
MUL = mybir.AluOpType.mult

_compiled = None


def _build(debug_dump=False):
    nc = bacc.Bacc("TRN2", target_bir_lowering=False, debug=False,
                   num_devices=N_CORES)

    xq_d = nc.dram_tensor("xq", [P, CQ_CH, TQC], BF16, kind="ExternalInput")
    xkv_d = nc.dram_tensor("xkv", [P, CKV_CH, TKV], BF16,
                           kind="ExternalInput")
    wq_d = nc.dram_tensor("wq", [P, CQ_CH, CQ], BF16, kind="ExternalInput")
    wk_d = nc.dram_tensor("wk", [P, CKV_CH, CQ], BF16, kind="ExternalInput")
    wv_d = nc.dram_tensor("wv", [P, CKV_CH, CQ], BF16, kind="ExternalInput")
    wo_d = nc.dram_tensor("wo", [CQ_CH, P, CQ], BF16, kind="ExternalInput")
    out_ext = nc.dram_tensor("out", [CQ, TQC], F32, kind="ExternalOutput")
    if debug_dump:
        dbg = {
            "dump_k": nc.dram_tensor("dump_k", [P, NHP, TKV], BF16,
                                     kind="ExternalOutput"),
            "dump_q": nc.dram_tensor("dump_q", [P, NHP, TQC], BF16,
                                     kind="ExternalOutput"),
            "dump_v": nc.dram_tensor("dump_v", [P, NI, NHP * 130], BF16,
                                     kind="ExternalOutput"),
            "dump_ao": nc.dram_tensor("dump_ao", [P, NHP, TQC], BF16,
                                      kind="ExternalOutput"),
        }

    with tile.TileContext(nc) as tc:
        with (
            tc.tile_pool(name="inp", bufs=1) as ipool,
            tc.tile_pool(name="kvq", bufs=1) as kpool,
            tc.tile_pool(name="stage", bufs=1) as stpool,
            tc.tile_pool(name="ps_sc", bufs=2, space="PSUM") as ps_sc,
            tc.tile_pool(name="ps_acc", bufs=2, space="PSUM") as ps_acc,
            tc.tile_pool(name="ps_pr", bufs=2, space="PSUM") as ps_pr,
        ):
            # ---- resident inputs / weights (bf16) ----
            xq_sb = ipool.tile([P, CQ_CH, TQC], BF16, tag="xq")
            xkv_sb = ipool.tile([P, CKV_CH, TKV], BF16, tag="xkv")
            wq_sb = ipool.tile([P, CQ_CH, CQ], BF16, tag="wq")
            wk_sb = ipool.tile([P, CKV_CH, CQ], BF16, tag="wk")
            wv_sb = ipool.tile([P, CKV_CH, CQ], BF16, tag="wv")
            warm = ipool.tile([P, 512], BF16, tag="warm")

            # ---- SBUF-resident projected tensors + attention output ----
            q_sb = kpool.tile([P, NHP, TQC], BF16, tag="q")
            k_sb = kpool.tile([P, NHP, TKV], BF16, tag="k")
            # per kv-chunk, per head-pair: [vA 64 | 1 | vB 64 | 1]
            v_sb = kpool.tile([P, NI, NHP * 130], BF16, tag="v")
            ao_sb = kpool.tile([P, NHP, TQC], BF16, tag="ao")

            # ---- input DMAs ----
            # The first k_unit/q_unit/v_unit deps are split across both
            # DMA queues (sync + gpsimd) to halve the time-to-first-matmul.
            xq_a, xkv_a = xq_d.ap(), xkv_d.ap()
            for c in range(CKV_CH):
                eng_w = nc.gpsimd if c < 3 else nc.sync
                eng_x = nc.sync if c < 3 else nc.gpsimd
                eng_w.dma_start(wk_sb[:, c, :], wk_d.ap()[:, c, :])
                eng_x.dma_start(xkv_sb[:, c, 0:512], xkv_a[:, c, 0:512])
            for c in range(CQ_CH):
                eng_w = nc.gpsimd if c < 4 else nc.sync
                eng_x = nc.sync if c < 4 else nc.gpsimd
                eng_w.dma_start(wq_sb[:, c, :], wq_d.ap()[:, c, :])
                eng_x.dma_start(xq_sb[:, c, 0:512], xq_a[:, c, 0:512])
            for c in range(CKV_CH):
                nc.gpsimd.dma_start(wv_sb[:, c, :], wv_d.ap()[:, c, :])
            for tb in range(1, 4):
                for c in range(CKV_CH):
                    nc.sync.dma_start(xkv_sb[:, c, tb * 512:(tb + 1) * 512],
                                      xkv_a[:, c, tb * 512:(tb + 1) * 512])
            for c in range(CQ_CH):
                nc.sync.dma_start(xq_sb[:, c, 512:1024], xq_a[:, c, 512:1024])
            # ones columns of v_sb (col 64 of every 65-wide sub-block)
            nc.gpsimd.memset(
                v_sb[:].rearrange("p n (c w) -> p n c w", w=65)[:, :, :, 64:65],
                1.0)

            # wo chunks stream in late (used only by the output projection)
            wo_ch = []
            for do in range(CQ_CH):
                w = stpool.tile([P, CQ], BF16, tag="wo", bufs=8,
                                name=f"wo_{do}")
                nc.gpsimd.dma_start(w[:], wo_d.ap()[do])
                wo_ch.append(w)

            # ---- PE warm-up while the first DMAs land ----
            nc.vector.memset(warm[:], 0.0)
            for w in range(12):
                pw = ps_pr.tile([P, 512], F32, tag="pr", name=f"warm_{w}")
                nc.tensor.matmul(pw[:], warm[:, 0:128], warm[:],
                                 start=True, stop=True)

            # ---- projection micro-units (emitted interleaved) ----
            def k_unit(hp, tb):
                # K for head-pair hp, 512-token block tb
                ts = slice(tb * 512, (tb + 1) * 512)
                hs = slice(hp * P, (hp + 1) * P)
                pk = ps_pr.tile([P, 512], F32, tag="pr", name=f"pk_{hp}_{tb}")
                for c in range(CKV_CH):
                    nc.tensor.matmul(pk[:], wk_sb[:, c, hs],
                                     xkv_sb[:, c, ts],
                                     start=(c == 0), stop=(c == CKV_CH - 1))
                nc.vector.tensor_copy(k_sb[:, hp, ts], pk[:])

            def q_unit(hp, jb):
                ts = slice(jb * 512, (jb + 1) * 512)
                hs = slice(hp * P, (hp + 1) * P)
                pq = ps_pr.tile([P, 512], F32, tag="pr", name=f"pq_{hp}_{jb}")
                for c in range(CQ_CH):
                    nc.tensor.matmul(pq[:], wq_sb[:, c, hs],
                                     xq_sb[:, c, ts],
                                     start=(c == 0), stop=(c == CQ_CH - 1))
                nc.vector.tensor_copy(q_sb[:, hp, ts], pq[:])

            def v_unit(tc_i, dvh):
                # V for kv-chunk tc_i, 512-wide dv half (4 head-pairs)
                ds = slice(dvh * 512, (dvh + 1) * 512)
                pv = ps_pr.tile([P, 512], F32, tag="pr",
                                name=f"pv_{tc_i}_{dvh}")
                for c in range(CKV_CH):
                    nc.tensor.matmul(
                        pv[:], xkv_sb[:, c, tc_i * P:(tc_i + 1) * P],
                        wv_sb[:, c, ds],
                        start=(c == 0), stop=(c == CKV_CH - 1))
                dst = v_sb[:, tc_i, dvh * 520:(dvh + 1) * 520]
                nc.vector.tensor_copy(
                    dst.rearrange("p (h c) -> p h c", c=65)[:, :, 0:64],
                    pv[:].rearrange("p (h d) -> p h d", d=64))

            o_live = {}

            def o_unit(j, do, lo, hi):
                # output projection partial chain (contraction chunks
                # lo..hi-1) for do-chunk do, query block j
                ts = slice(j * 512, (j + 1) * 512)
                if lo == 0:
                    po = ps_pr.tile([P, 512], F32, tag="pr",
                                    name=f"po_{j}_{do}")
                    o_live[(j, do)] = po
                else:
                    po = o_live[(j, do)]
                for n in range(lo, hi):
                    nc.tensor.matmul(po[:], wo_ch[do][:, n * P:(n + 1) * P],
                                     ao_sb[:, n, ts],
                                     start=(n == 0), stop=(n == CQ_CH - 1))
                if hi == CQ_CH:
                    ost = stpool.tile([P, 512], F32, tag="ost", bufs=2)
                    nc.vector.tensor_copy(ost[:], po[:])
                    nc.sync.dma_start(
                        out_ext[do * P:(do + 1) * P, ts], ost[:])
                    del o_live[(j, do)]

            # minimal up-front projections: first K/Q/V pieces of block
            # (0,0); the rest of head-pair 0 interleaves into the block
            k_unit(0, 0)
            q_unit(0, 0)
            v_unit(0, 0)
            v_unit(0, 1)

            # deferred units: K/Q for hp 1..7, then out-proj chains
            units = []
            for hp in range(1, NHP):
                for tb in range(4):
                    units.append(lambda hp=hp, tb=tb: k_unit(hp, tb))
                for jb in range(NJ):
                    units.append(lambda hp=hp, jb=jb: q_unit(hp, jb))
            n_proj_units = len(units)  # 42
            for j in range(NJ):
                for do in range(CQ_CH):
                    for (lo, hi) in ((0, 3), (3, 6), (6, CQ_CH)):
                        units.append(lambda j=j, do=do, lo=lo, hi=hi:
                                     o_unit(j, do, lo, hi))
            n_oj = 24  # o-units per j-block
            upos = 0

            def drain_to(target):
                nonlocal upos
                target = min(target, len(units))
                while upos < target:
                    units[upos]()
                    upos += 1

            # ================= attention =================
            for hp in range(NHP):
                for j in range(NJ):
                    js = slice(j * 512, (j + 1) * 512)
                    acc = [ps_acc.tile([P, 512], F32, tag="acc",
                                       name=f"acc_{hp}_{j}_{g}")
                           for g in range(2)]
                    # unit budget this block must reach by its end:
                    # hp h's K/Q must be complete before block (h, 0);
                    # out-proj j0 drains in the final block, j1 in the tail
                    if hp == NHP - 1 and j == 1:
                        target = n_proj_units + n_oj
                    else:
                        target = min(n_proj_units, 6 * hp + 3 * (j + 1))
                    start_pos = upos
                    for i in range(NI):
                        isl = slice(i * P, (i + 1) * P)
                        sc = ps_sc.tile([P, 1024], F32, tag="sc")
                        nc.tensor.matmul(sc[:, 0:512],
                                         k_sb[0:64, hp, isl],
                                         q_sb[0:64, hp, js],
                                         start=True, stop=True)
                        nc.tensor.matmul(sc[:, 512:1024],
                                         k_sb[64:128, hp, isl],
                                         q_sb[64:128, hp, js],
                                         start=True, stop=True,
                                         tile_position=(64, 0))
                        ex = stpool.tile([P, 1024], BF16, tag="ex", bufs=4)
                        nc.scalar.activation(ex[:], sc[:], EXP, scale=0.125)
                        # interleave work between the scores and their PV
                        # consumers: fills the PE while exp runs, and at
                        # block starts hides the accumulator hand-off
                        if hp == 0 and j == 0:
                            # produce V one kv-chunk ahead of the PV reads,
                            # plus the rest of head-pair 0's K/Q (sc for
                            # chunk 4tb needs k_unit(0,tb), emitted 2
                            # chunks ahead)
                            if i < NI - 1:
                                v_unit(i + 1, 0)
                                v_unit(i + 1, 1)
                            if i in (2, 6, 10):
                                k_unit(0, i // 4 + 1)
                            if i == 12:
                                q_unit(0, 1)
                        else:
                            frac = start_pos + \
                                ((i + 1) * (target - start_pos)) // NI
                            front = start_pos + 3 if i == 0 else 0
                            drain_to(max(frac, front))
                        # PV: scores stationary [kv,128q], V moving [kv,65].
                        # One accumulation group per acc tile (= one PSUM
                        # zero region): start marks the whole bank pending-
                        # zero, the four qc sub-chains then overwrite their
                        # slots at i==0 and accumulate afterwards.
                        for g in range(2):
                            for qc in range(4):
                                vcol = hp * 130 + g * 65
                                nc.tensor.matmul(
                                    acc[g][:, qc * P:qc * P + 65],
                                    ex[:, g * 512 + qc * P:
                                       g * 512 + (qc + 1) * P],
                                    v_sb[:, i, vcol:vcol + 65],
                                    start=(i == 0 and qc == 0),
                                    stop=(i == NI - 1 and qc == 3))
                    drain_to(target)
                    # normalise + transpose into ao (scale split across
                    # DVE and Pool so the accumulator banks free quickly)
                    for qc in range(4):
                        rec = stpool.tile([P, 2], F32, tag="rec", bufs=4)
                        aoq = stpool.tile([P, 128], BF16, tag="aoq", bufs=4)
                        for g in range(2):
                            nc.vector.reciprocal(
                                rec[:, g:g + 1],
                                acc[g][:, qc * P + 64:qc * P + 65])
                            eng = nc.vector if g == 0 else nc.gpsimd
                            eng.tensor_scalar(
                                aoq[:, g * 64:(g + 1) * 64],
                                acc[g][:, qc * P:qc * P + 64],
                                rec[:, g:g + 1], None, op0=MUL)
                        nc.sync.dma_start_transpose(
                            ao_sb[:, hp, j * 512 + qc * P:
                                  j * 512 + (qc + 1) * P],
                            aoq[:])

            # tail: remaining out-proj units (query block j1)
            drain_to(len(units))

            if debug_dump:
                nc.sync.dma_start(dbg["dump_k"].ap(), k_sb[:])
                nc.sync.dma_start(dbg["dump_q"].ap(), q_sb[:])
                nc.sync.dma_start(dbg["dump_v"].ap(), v_sb[:])
                nc.sync.dma_start(dbg["dump_ao"].ap(), ao_sb[:])

    nc.compile()
    return nc


def make_in_maps(q_tokens, kv_tokens, Wq, Wk, Wv, Wo):
    import ml_dtypes
    bf16 = ml_dtypes.bfloat16
    q_tokens = np.asarray(q_tokens, np.float32)
    kv_tokens = np.asarray(kv_tokens, np.float32)
    Wq = np.asarray(Wq, np.float32)
    Wk = np.asarray(Wk, np.float32)
    Wv = np.asarray(Wv, np.float32)
    Wo = np.asarray(Wo, np.float32)

    # [p, n, t] = x[t, n*128+p]
    def chan_major(x, nch):
        return np.ascontiguousarray(
            x.T.reshape(nch, P, -1).transpose(1, 0, 2)).astype(bf16)

    xkv_h = [chan_major(kv_tokens[b], CKV_CH) for b in range(4)]
    wq_h = chan_major(Wq, CQ_CH)   # wq[p, n, d] = Wq[d, n*128+p]
    wk_h = chan_major(Wk, CKV_CH)
    wv_h = chan_major(Wv, CKV_CH)
    # wo[do_ch, p, n*128 + do_in] = Wo[do_ch*128 + do_in, n*128 + p]
    wo_h = np.ascontiguousarray(
        Wo.reshape(CQ_CH, P, CQ_CH, P).transpose(0, 3, 2, 1)
        .reshape(CQ_CH, P, CQ)).astype(bf16)

    in_maps = []
    for c in range(N_CORES):
        b, qh = c // 2, c % 2
        xq = chan_major(q_tokens[b, qh * TQC:(qh + 1) * TQC], CQ_CH)
        in_maps.append({
            "xq": xq,
            "xkv": xkv_h[b],
            "wq": wq_h,
            "wk": wk_h,
            "wv": wv_h,
            "wo": wo_h,
        })
    return in_maps


def kernel(q_tokens, kv_tokens, Wq, Wk, Wv, Wo):
    global _compiled
    if _compiled is None:
        _compiled = _build()
    nc = _compiled

    in_maps = make_in_maps(q_tokens, kv_tokens, Wq, Wk, Wv, Wo)
    res = bass_utils.run_bass_kernel_spmd(nc, in_maps,
                                          core_ids=list(range(N_CORES)))
    B = 4
    out = np.empty((B, 2048, CQ), np.float32)
    for c in range(N_CORES):
        b, qh = c // 2, c % 2
        out[b, qh * TQC:(qh + 1) * TQC, :] = res.results[c]["out"].T
    return out


# revision 28
# speedup vs baseline: 1.0403x; 1.0403x over previous
"""Cross-attention kernel for 8 Trainium2 NeuronCores (SPMD).

Problem: B=4, T_q=T_kv=2048, Q_DIM=1024, KV_DIM=768, H=16, DK=64, fp32.
  q = q_tokens @ Wq.T ; k = kv_tokens @ Wk.T ; v = kv_tokens @ Wv.T
  out = softmax(q k^T / sqrt(DK)) v @ Wo.T

Sharding (8 cores): core c handles batch b=c//2 and q-token half
qh=c%2 (1024 of the 2048 query tokens) with ALL 16 heads resident.
No collectives at all: the output projection contracts over the full
1024 attention-output dims locally, and the host concatenates the
per-core [1024 tokens, 1024 dims] blocks.  (K/V projections are
recomputed per batch-pair — far cheaper than pair AllGathers, whose
~15us setup at ~40GB/s would dominate the whole kernel.)

Everything stays in SBUF between phases (no K/V/Q DRAM round-trips):
inputs and weights stream in as bf16; Q/K/V are projected into
SBUF-resident bf16 tensors; attention runs per head-pair out of SBUF.
All matmuls take bf16 operands with fp32 PSUM accumulation.

Attention inner loop per (head-pair hp, 512-query block j, kv-chunk i):
scores for the pair's two heads go to the two 64-row halves of the PE
array (row tiling via base_partition/tile_position); exp runs on the
scalar engine (the only engine with activation tables — it is the
attention-phase bottleneck at ~1.04us per [128,1024] tile, so all DMA
issue is kept off it); the PV matmul is reoriented (exp'd scores
stationary [kv,128q], V moving [kv,65]) which halves its PE rows vs the
classic orientation, with the softmax denominator accumulated for free
by a ones-column appended to V.  Normalisation is a per-partition
reciprocal+scale on DVE (no partition broadcast), and the [q,dv]->[dv,q]
transpose rides the DMA xbar (dma_start_transpose) instead of the PE.

The remaining head-pairs' K/Q projections are interleaved unit-by-unit
into the attention stream so the PE fills the slack of the Act-bound
phase (front-loaded at block starts, which also hides the single-
buffered PSUM accumulator hand-off), and the output projection for the
first query block interleaves into the last attention block.

PSUM (8 banks): scores [P,1024]x2 = 4, accumulators [P,512]x2 = 2
(four 65-wide PV chains at 128-col slots per tile), projection/warm-up/
out-proj chains [P,512]x2 = 2.
"""

import numpy as np

import concourse.bacc as bacc
import concourse.mybir as mybir
import concourse.tile as tile
from concourse import bass_utils

N_CORES = 8
P = 128
TQC = 1024     # per-core query tokens
TKV = 2048
CQ = 1024      # q_tokens channels / q dims / output dims
CKV = 768      # kv_tokens channels
NHP = 8        # head-pairs (128 dims each)
NI = TKV // P  # 16 kv chunks
NJ = TQC // 512  # 2 query blocks
CQ_CH = CQ // P   # 8
CKV_CH = CKV // P  # 6

F32 = mybir.dt.float32
BF16 = mybir.dt.bfloat16
EXP = mybir.ActivationFunctionType.Exp
MUL = mybir.AluOpType.mult

_compiled = None


def _build(debug_dump=False):
    nc = bacc.Bacc("TRN2", target_bir_lowering=False, debug=False,
                   num_devices=N_CORES)

    xq_d = nc.dram_tensor("xq", [P, CQ_CH, TQC], BF16, kind="ExternalInput")
    xkv_d = nc.dram_tensor("xkv", [P, CKV_CH, TKV], BF16,
                           kind="ExternalInput")
    wq_d = nc.dram_tensor("wq", [P, CQ_CH, CQ], BF16, kind="ExternalInput")
    wk_d = nc.dram_tensor("wk", [P, CKV_CH, CQ], BF16, kind="ExternalInput")
    wv_d = nc.dram_tensor("wv", [P, CKV_CH, CQ], BF16, kind="ExternalInput")
    wo_d = nc.dram_tensor("wo", [CQ_CH, P, CQ], BF16, kind="ExternalInput")
    out_ext = nc.dram_tensor("out", [CQ, TQC], F32, kind="ExternalOutput")
    if debug_dump:
        dbg = {
            "dump_k": nc.dram_tensor("dump_k", [P, NHP, TKV], BF16,
                                     kind="ExternalOutput"),
            "dump_q": nc.dram_tensor("dump_q", [P, NHP, TQC], BF16,
                                     kind="ExternalOutput"),
            "dump_v": nc.dram_tensor("dump_v", [P, NI, NHP * 130], BF16,
                                     kind="ExternalOutput"),
            "dump_ao": nc.dram_tensor("dump_ao", [P, NHP, TQC], BF16,
                                      kind="ExternalOutput"),
        }

    with tile.TileContext(nc) as tc:
        with (
            tc.tile_pool(name="inp", bufs=1) as ipool,
            tc.tile_pool(name="kvq", bufs=1) as kpool,
            tc.tile_pool(name="stage", bufs=1) as stpool,
            tc.tile_pool(name="ps_sc", bufs=2, space="PSUM") as ps_sc,
            tc.tile_pool(name="ps_acc", bufs=2, space="PSUM") as ps_acc,
            tc.tile_pool(name="ps_pr", bufs=2, space="PSUM") as ps_pr,
        ):
            # ---- resident inputs / weights (bf16) ----
            xq_sb = ipool.tile([P, CQ_CH, TQC], BF16, tag="xq")
            xkv_sb = ipool.tile([P, CKV_CH, TKV], BF16, tag="xkv")
            wq_sb = ipool.tile([P, CQ_CH, CQ], BF16, tag="wq")
            wk_sb = ipool.tile([P, CKV_CH, CQ], BF16, tag="wk")
            wv_sb = ipool.tile([P, CKV_CH, CQ], BF16, tag="wv")
            warm = ipool.tile([P, 512], BF16, tag="warm")

            # ---- SBUF-resident projected tensors + attention output ----
            q_sb = kpool.tile([P, NHP, TQC], BF16, tag="q")
            k_sb = kpool.tile([P, NHP, TKV], BF16, tag="k")
            # per kv-chunk, per head-pair: [vA 64 | 1 | vB 64 | 1]
            v_sb = kpool.tile([P, NI, NHP * 130], BF16, tag="v")
            ao_sb = kpool.tile([P, NHP, TQC], BF16, tag="ao")

            # ---- input DMAs ----
            # The first k_unit/q_unit/v_unit deps are split across both
            # DMA queues (sync + gpsimd) to halve the time-to-first-matmul.
            xq_a, xkv_a = xq_d.ap(), xkv_d.ap()
            for c in range(CKV_CH):
                eng_w = nc.gpsimd if c < 3 else nc.sync
                eng_x = nc.sync if c < 3 else nc.gpsimd
                eng_w.dma_start(wk_sb[:, c, :], wk_d.ap()[:, c, :])
                eng_x.dma_start(xkv_sb[:, c, 0:512], xkv_a[:, c, 0:512])
            for c in range(CQ_CH):
                eng_w = nc.gpsimd if c < 4 else nc.sync
                eng_x = nc.sync if c < 4 else nc.gpsimd
                eng_w.dma_start(wq_sb[:, c, :], wq_d.ap()[:, c, :])
                eng_x.dma_start(xq_sb[:, c, 0:512], xq_a[:, c, 0:512])
            for c in range(CKV_CH):
                nc.gpsimd.dma_start(wv_sb[:, c, :], wv_d.ap()[:, c, :])
            for tb in range(1, 4):
                for c in range(CKV_CH):
                    nc.sync.dma_start(xkv_sb[:, c, tb * 512:(tb + 1) * 512],
                                      xkv_a[:, c, tb * 512:(tb + 1) * 512])
            for c in range(CQ_CH):
                nc.sync.dma_start(xq_sb[:, c, 512:1024], xq_a[:, c, 512:1024])
            # ones columns of v_sb (col 64 of every 65-wide sub-block)
            nc.gpsimd.memset(
                v_sb[:].rearrange("p n (c w) -> p n c w", w=65)[:, :, :, 64:65],
                1.0)

            # wo chunks stream in late (used only by the output projection)
            wo_ch = []
            for do in range(CQ_CH):
                w = stpool.tile([P, CQ], BF16, tag="wo", bufs=8,
                                name=f"wo_{do}")
                nc.gpsimd.dma_start(w[:], wo_d.ap()[do])
                wo_ch.append(w)

            # ---- PE warm-up while the first DMAs land ----
            nc.vector.memset(warm[:], 0.0)
            for w in range(12):
                pw = ps_pr.tile([P, 512], F32, tag="pr", name=f"warm_{w}")
                nc.tensor.matmul(pw[:], warm[:, 0:128], warm[:],
                                 start=True, stop=True)

            # ---- projection micro-units (emitted interleaved) ----
            def k_unit(hp, tb):
                # K for head-pair hp, 512-token block tb
                ts = slice(tb * 512, (tb + 1) * 512)
                hs = slice(hp * P, (hp + 1) * P)
                pk = ps_pr.tile([P, 512], F32, tag="pr", name=f"pk_{hp}_{tb}")
                for c in range(CKV_CH):
                    nc.tensor.matmul(pk[:], wk_sb[:, c, hs],
                                     xkv_sb[:, c, ts],
                                     start=(c == 0), stop=(c == CKV_CH - 1))
                nc.vector.tensor_copy(k_sb[:, hp, ts], pk[:])

            def q_unit(hp, jb):
                ts = slice(jb * 512, (jb + 1) * 512)
                hs = slice(hp * P, (hp + 1) * P)
                pq = ps_pr.tile([P, 512], F32, tag="pr", name=f"pq_{hp}_{jb}")
                for c in range(CQ_CH):
                    nc.tensor.matmul(pq[:], wq_sb[:, c, hs],
                                     xq_sb[:, c, ts],
                                     start=(c == 0), stop=(c == CQ_CH - 1))
                nc.vector.tensor_copy(q_sb[:, hp, ts], pq[:])

            def v_unit(tc_i, dvh):
                # V for kv-chunk tc_i, 512-wide dv half (4 head-pairs)
                ds = slice(dvh * 512, (dvh + 1) * 512)
                pv = ps_pr.tile([P, 512], F32, tag="pr",
                                name=f"pv_{tc_i}_{dvh}")
                for c in range(CKV_CH):
                    nc.tensor.matmul(
                        pv[:], xkv_sb[:, c, tc_i * P:(tc_i + 1) * P],
                        wv_sb[:, c, ds],
                        start=(c == 0), stop=(c == CKV_CH - 1))
                dst = v_sb[:, tc_i, dvh * 520:(dvh + 1) * 520]
                nc.vector.tensor_copy(
                    dst.rearrange("p (h c) -> p h c", c=65)[:, :, 0:64],
                    pv[:].rearrange("p (h d) -> p h d", d=64))

            o_live = {}

            def o_unit(j, do, lo, hi, qb=None, pool=None, store=None):
                # output projection partial chain (contraction chunks
                # lo..hi-1) for do-chunk do; qb=None covers the whole
                # 512-query block j, qb=0/1 a 256-query half (used for
                # the tail so only the last quarter serialises).  Tail
                # chains alternate between the pr and the by-then-idle
                # acc PSUM pools, and their stores alternate DMA queues.
                if qb is None:
                    ts = slice(j * 512, (j + 1) * 512)
                    w = 512
                else:
                    ts = slice(j * 512 + qb * 256, j * 512 + (qb + 1) * 256)
                    w = 256
                key = (j, do, qb)
                if lo == 0:
                    pl, tg = (ps_pr, "pr") if pool is None else pool
                    po = pl.tile([P, 512], F32, tag=tg,
                                 name=f"po_{j}_{do}_{qb}")
                    o_live[key] = po
                else:
                    po = o_live[key]
                for n in range(lo, hi):
                    nc.tensor.matmul(po[:, 0:w],
                                     wo_ch[do][:, n * P:(n + 1) * P],
                                     ao_sb[:, n, ts],
                                     start=(n == 0), stop=(n == CQ_CH - 1))
                if hi == CQ_CH:
                    ost = stpool.tile([P, 512], F32, tag="ost", bufs=2)
                    nc.vector.tensor_copy(ost[:, 0:w], po[:, 0:w])
                    (store or nc.sync).dma_start(
                        out_ext[do * P:(do + 1) * P, ts], ost[:, 0:w])
                    del o_live[key]

            # minimal up-front projections: first K/Q/V pieces of block
            # (0,0); the rest of head-pair 0 interleaves into the block.
            # Only the dvh=0 half of V (head-pairs 0-3) is produced in
            # block (0,0); the dvh=1 half is deferred into the drain
            # stream (needed first by block (4,0)).
            k_unit(0, 0)
            q_unit(0, 0)

            # deferred units: K/Q for hp 1..7 with V-half-1 after hp3,
            # then out-proj chains
            units = []
            for hp in range(1, NHP):
                for tb in range(4):
                    units.append(lambda hp=hp, tb=tb: k_unit(hp, tb))
                for jb in range(NJ):
                    units.append(lambda hp=hp, jb=jb: q_unit(hp, jb))
                if hp == 3:
                    for tc_i in range(NI):
                        units.append(
                            lambda tc_i=tc_i: v_unit(tc_i, 1))
            n_proj_units = len(units)  # 58
            # out-proj j0 at full width (drained inside block (7,1)),
            # j1 in 256-query halves so the tail only waits on the last
            # quarter of the final block's normalisation
            for do in range(CQ_CH):
                for (lo, hi) in ((0, 3), (3, 6), (6, CQ_CH)):
                    units.append(lambda do=do, lo=lo, hi=hi:
                                 o_unit(0, do, lo, hi))
            # j1 in 256-query halves, software-pipelined 2-deep: the
            # n=0..3 partials (which need no hp4-7 j1 data) run ahead of
            # the n=4..7 finals so the in-order PE doesn't stall on the
            # final block's normalisation latency
            for qb in range(2):
                def _part(do, qb):
                    pool = (ps_pr, "pr") if do % 2 == 0 else (ps_acc, "acc")
                    o_unit(1, do, 0, 4, qb=qb, pool=pool)

                def _final(do, qb):
                    store = nc.sync if do % 2 == 0 else nc.gpsimd
                    o_unit(1, do, 4, CQ_CH, qb=qb, store=store)
                parts = [(lambda do=do, qb=qb: _part(do, qb))
                         for do in range(CQ_CH)]
                finals = [(lambda do=do, qb=qb: _final(do, qb))
                          for do in range(CQ_CH)]
                pipe = []
                for idx in range(CQ_CH):
                    pipe.append(parts[idx])
                    if idx >= 3:
                        pipe.append(finals[idx - 3])
                pipe.extend(finals[CQ_CH - 3:])
                units.extend(pipe)
            upos = 0

            def drain_to(target):
                nonlocal upos
                target = min(target, len(units))
                while upos < target:
                    units[upos]()
                    upos += 1

            # ================= attention =================
            # Block order: hp0-3 interleave j0/j1; hp4-7 run all j0
            # blocks first so the j0 output projection becomes available
            # early and drains inside the Act-bound slack of the last
            # four blocks — only the j1 out-proj remains for the tail.
            border = [(0, 0), (0, 1), (1, 0), (1, 1), (2, 0), (2, 1),
                      (3, 0), (3, 1), (4, 0), (5, 0), (6, 0), (7, 0),
                      (4, 1), (5, 1), (6, 1), (7, 1)]
            # cumulative unit-drain target by the END of each block
            btgt = [0, 6, 11, 16, 21, 26, 32, 40,
                    46, 52, 58, 58, 64, 70, 76, 82]
            for bi, (hp, j) in enumerate(border):
                    js = slice(j * 512, (j + 1) * 512)
                    acc = [ps_acc.tile([P, 512], F32, tag="acc",
                                       name=f"acc_{hp}_{j}_{g}")
                           for g in range(2)]
                    target = btgt[bi]
                    start_pos = upos
                    exs = []

                    def emit_pv(i, ex, hp=hp, acc=acc):
                        for g in range(2):
                            for qc in range(4):
                                vcol = hp * 130 + g * 65
                                nc.tensor.matmul(
                                    acc[g][:, qc * P:qc * P + 65],
                                    ex[:, g * 512 + qc * P:
                                       g * 512 + (qc + 1) * P],
                                    v_sb[:, i, vcol:vcol + 65],
                                    start=(i == 0 and qc == 0),
                                    stop=(i == NI - 1 and qc == 3))
                    for i in range(NI):
                        isl = slice(i * P, (i + 1) * P)
                        sc = ps_sc.tile([P, 1024], F32, tag="sc")
                        nc.tensor.matmul(sc[:, 0:512],
                                         k_sb[0:64, hp, isl],
                                         q_sb[0:64, hp, js],
                                         start=True, stop=True)
                        nc.tensor.matmul(sc[:, 512:1024],
                                         k_sb[64:128, hp, isl],
                                         q_sb[64:128, hp, js],
                                         start=True, stop=True,
                                         tile_position=(64, 0))
                        ex = stpool.tile([P, 1024], BF16, tag="ex", bufs=4)
                        nc.scalar.activation(ex[:], sc[:], EXP, scale=0.125)
                        # interleave work between the scores and their PV
                        # consumers: fills the PE while exp runs, and at
                        # block starts hides the accumulator hand-off
                        if hp == 0 and j == 0:
                            # produce the dvh=0 half of V (all block 0
                            # needs) one kv-chunk ahead of the PV reads,
                            # plus the rest of head-pair 0's K/Q (sc for
                            # chunk 4tb needs k_unit(0,tb), emitted 2
                            # chunks ahead)
                            if i == 0:
                                v_unit(0, 0)
                            if i < NI - 1:
                                v_unit(i + 1, 0)
                            if i in (2, 6, 10):
                                k_unit(0, i // 4 + 1)
                            if i == 12:
                                q_unit(0, 1)
                        else:
                            frac = start_pos + \
                                ((i + 1) * (target - start_pos)) // NI
                            front = start_pos + 3 if i == 0 else 0
                            drain_to(min(max(frac, front), target))
                        # PV: scores stationary [kv,128q], V moving [kv,65],
                        # emitted one iteration behind the scores so the
                        # in-order PE never waits on the exp latency.
                        # One accumulation group per acc tile (= one PSUM
                        # zero region): start marks the whole bank pending-
                        # zero, the four qc sub-chains then overwrite their
                        # slots at i==0 and accumulate afterwards.
                        exs.append((i, ex))
                        if len(exs) > 1:
                            emit_pv(*exs.pop(0))
                    emit_pv(*exs.pop(0))
                    drain_to(target)
                    # normalise + transpose into ao (scale split across
                    # DVE and Pool so the accumulator banks free quickly)
                    for qc in range(4):
                        rec = stpool.tile([P, 2], F32, tag="rec", bufs=4)
                        aoq = stpool.tile([P, 128], BF16, tag="aoq", bufs=4)
                        # (normalisation stays on DVE: GPSIMD cannot
                        # access PSUM on TRN2 — the compiler rejects it)
                        for g in range(2):
                            nc.vector.reciprocal(
                                rec[:, g:g + 1],
                                acc[g][:, qc * P + 64:qc * P + 65])
                            nc.vector.tensor_scalar(
                                aoq[:, g * 64:(g + 1) * 64],
                                acc[g][:, qc * P:qc * P + 64],
                                rec[:, g:g + 1], None, op0=MUL)
                        # the final block's transposes ride the scalar
                        # queue (idle after the last exp) so they don't
                        # queue behind the out-proj stores on sync
                        teng = nc.scalar if bi == len(border) - 1 else nc.sync
                        teng.dma_start_transpose(
                            ao_sb[:, hp, j * 512 + qc * P:
                                  j * 512 + (qc + 1) * P],
                            aoq[:])

            # tail: remaining out-proj units (query block j1)
            drain_to(len(units))

            if debug_dump:
                nc.sync.dma_start(dbg["dump_k"].ap(), k_sb[:])
                nc.sync.dma_start(dbg["dump_q"].ap(), q_sb[:])
                nc.sync.dma_start(dbg["dump_v"].ap(), v_sb[:])
                nc.sync.dma_start(dbg["dump_ao"].ap(), ao_sb[:])

    nc.compile()
    return nc


def make_in_maps(q_tokens, kv_tokens, Wq, Wk, Wv, Wo):
    import ml_dtypes
    bf16 = ml_dtypes.bfloat16
    q_tokens = np.asarray(q_tokens, np.float32)
    kv_tokens = np.asarray(kv_tokens, np.float32)
    Wq = np.asarray(Wq, np.float32)
    Wk = np.asarray(Wk, np.float32)
    Wv = np.asarray(Wv, np.float32)
    Wo = np.asarray(Wo, np.float32)

    # [p, n, t] = x[t, n*128+p]
    def chan_major(x, nch):
        return np.ascontiguousarray(
            x.T.reshape(nch, P, -1).transpose(1, 0, 2)).astype(bf16)

    xkv_h = [chan_major(kv_tokens[b], CKV_CH) for b in range(4)]
    wq_h = chan_major(Wq, CQ_CH)   # wq[p, n, d] = Wq[d, n*128+p]
    wk_h = chan_major(Wk, CKV_CH)
    wv_h = chan_major(Wv, CKV_CH)
    # wo[do_ch, p, n*128 + do_in] = Wo[do_ch*128 + do_in, n*128 + p]
    wo_h = np.ascontiguousarray(
        Wo.reshape(CQ_CH, P, CQ_CH, P).transpose(0, 3, 2, 1)
        .reshape(CQ_CH, P, CQ)).astype(bf16)

    in_maps = []
    for c in range(N_CORES):
        b, qh = c // 2, c % 2
        xq = chan_major(q_tokens[b, qh * TQC:(qh + 1) * TQC], CQ_CH)
        in_maps.append({
            "xq": xq,
            "xkv": xkv_h[b],
            "wq": wq_h,
            "wk": wk_h,
            "wv": wv_h,
            "wo": wo_h,
        })
    return in_maps


def kernel(q_tokens, kv_tokens, Wq, Wk, Wv, Wo):
    global _compiled
    if _compiled is None:
        _compiled = _build()
    nc = _compiled

    in_maps = make_in_maps(q_tokens, kv_tokens, Wq, Wk, Wv, Wo)
    res = bass_utils.run_bass_kernel_spmd(nc, in_maps,
                                          core_ids=list(range(N_CORES)))
    B = 4
    out = np.empty((B, 2048, CQ), np.float32)
    for c in range(N_CORES):
        b, qh = c // 2, c % 2
        out[b, qh * TQC:(qh + 1) * TQC, :] = res.results[c]["out"].T
    return out
